# revision 11
# baseline (speedup 1.0000x reference)
"""Trainium2 Bass kernel for nn_Attention_82403242541756 (v2).

Reference semantics (with the dim-0 chunk bug):
  qkv = inputs @ W_qkv + b_qkv                  # [3, 2048, 3072]
  q, k, v = split(qkv, 3, axis=0)               # batch split! q=batch0, k=batch1, v=batch2
  each chunk [1, 2048, 3072] flat-reinterpreted to (3, 16, 2048, 64) = 48 "heads"
  scores softmax (no max needed; |scores*scale| < ~2.3), ctx, flat-reinterpret, @ W_out + b_out

Sharding (zero communication): core c takes seq rows [256c, 256c+256) of all 3
batch items -> local heads g in [6c, 6c+6), and final output rows [768c, 768c+768).

v2 design vs v1:
  - scores matmuls in fp8e4 DoubleRow (stride-0 duplicated 64-d halves -> 2x in
    the PE cost model; psum scores are 2*q.k so ACT exp uses scale/2)
  - ctx matmuls flipped to [s, d] orientation: lhsT = expT stride-16 s-columns,
    rhs = vx [t, 65] -> out free dim 65 instead of 512-chunks (2x cheaper), and
    the normalized X tile lands directly in ctx2d row layout.
  - denominators via N=1 matmuls into a dedicated psum bank.
  - out-projection: X [128 rows, 1024] -> DRAM -> 8 xbar-transpose readbacks ->
    out^T accumulation with full K=128 contraction.
  - PSUM budget: scores 4 banks (4 x [128,512] slots) + ctx 2 + den 1 +
    shared qkv-evac/outproj 1 = 8.
  - software-pipelined emission: QKV m1 and out-projections ride the PE slack
    under the ACT-bound attention windows.
"""

import sys

sys.path.insert(0, "/opt/trn_rl_repo")

import numpy as np
import ml_dtypes

from concourse import bacc, bass, mybir, tile
from concourse.bass_utils import run_bass_kernel_spmd

BF16 = mybir.dt.bfloat16
F32 = mybir.dt.float32
FP8 = mybir.dt.float8e4
AF = mybir.ActivationFunctionType
ALU = mybir.AluOpType
PM = mybir.MatmulPerfMode

P = 128
N_CORES = 8
SEQ = 2048
H = 1024
HEADS_PER_CORE = 6
ROWS = 256  # seq rows per core
SCALE = float(H) ** -0.5  # 1/32
# scores psum holds 2*q.k (DoubleRow duplicated halves) -> fold the 0.5 here
ACT_SCALE = SCALE / 2.0

_NC_CACHE = {}


def _build():
    nc = bacc.Bacc()

    xt_e = nc.declare_dram_parameter("xt", [P, 8, 768], BF16, isOutput=False)
    wq_e = nc.declare_dram_parameter("wq", [P, 8, 3072], BF16, isOutput=False)
    bq_e = nc.declare_dram_parameter("bq", [P, 3072], F32, isOutput=False)
    wo_e = nc.declare_dram_parameter("wo", [P, 8, 1024], BF16, isOutput=False)
    bo_e = nc.declare_dram_parameter("bo", [P, 8], F32, isOutput=False)
    out_e = nc.declare_dram_parameter("outt", [1024, 768], F32, isOutput=True)

    with tile.TileContext(nc) as tc:
        with (
            tc.tile_pool(name="dram", bufs=1, space="DRAM") as dp,
            tc.tile_pool(name="w1", bufs=1) as w1p,
            tc.tile_pool(name="scps", bufs=1, space="PSUM") as scps_p,
            tc.tile_pool(name="ctxps", bufs=1, space="PSUM") as ctxps_p,
            tc.tile_pool(name="denps", bufs=1, space="PSUM") as denps_p,
            tc.tile_pool(name="yps", bufs=1, space="PSUM") as yps_p,
            tc.tile_pool(name="yb", bufs=4) as ybp,
            tc.tile_pool(name="qk", bufs=4) as qkp,
            tc.tile_pool(name="q8", bufs=4) as q8p,
            tc.tile_pool(name="vx", bufs=2) as vxp,
            tc.tile_pool(name="expp", bufs=3) as expp,
            tc.tile_pool(name="xs", bufs=2) as xsp,
            tc.tile_pool(name="xt2", bufs=2) as xtp,
            tc.tile_pool(name="rs", bufs=2) as rsp,
            tc.tile_pool(name="stg", bufs=2) as stgp,
        ):
            # DRAM staging: yq/yk padded to 128 cols for the xbar transpose
            # (pad cols never written; transposed garbage lands on unused
            # partitions 64:128 and is never consumed).
            yq = dp.tile([12288, P], BF16)
            yk = dp.tile([12288, P], BF16)
            yv = dp.tile([12288, 64], BF16)
            yq_v = yq.rearrange("(r j) d -> r j d", j=48)
            yk_v = yk.rearrange("(r j) d -> r j d", j=48)
            yv_v = yv.rearrange("(r j) d -> r j d", j=48)
            xd = [dp.tile([P, 1024], BF16, name=f"xd{l}") for l in range(6)]

            # persistent weights / biases
            xt_sb = w1p.tile([P, 8, 768], BF16)
            wq_sb = w1p.tile([P, 8, 3072], BF16)
            bq_sb = w1p.tile([P, 3072], F32)
            wo_sb = w1p.tile([P, 8, 1024], BF16)
            bo_sb = w1p.tile([P, 8], F32)

            # ACT exp-table warmup: absorb the table load before real work
            warm = w1p.tile([P, 16], F32)
            nc.vector.memset(warm[:], 0.0)
            warm2 = w1p.tile([P, 16], F32)
            nc.scalar.activation(warm2[:], warm[:], AF.Exp, scale=1.0)

            nc.scalar.dma_start(xt_sb[:], xt_e[:])
            # wq streamed nb-major (chain consumption order)
            for nb in range(6):
                eng = nc.sync if nb % 2 == 0 else nc.scalar
                eng.dma_start(
                    wq_sb[:, :, 512 * nb : 512 * (nb + 1)],
                    wq_e[:, :, 512 * nb : 512 * (nb + 1)],
                )
            nc.scalar.dma_start(bq_sb[:], bq_e[:])
            nc.sync.dma_start(wo_sb[:], wo_e[:])
            nc.sync.dma_start(bo_sb[:], bo_e[:])

            # persistent psum: 4 score slots + ctx + den (+ shared yps/ops)
            sc = scps_p.tile([P, 4, 512], F32)
            ctxps = ctxps_p.tile([P, 16, 64], F32)
            denps = denps_p.tile([P, 16], F32)

            # ---------------- QKV chains ----------------
            # chain (b, m, nb): rows [b*256+128m, +128), qkv cols [512nb, +512)
            def emit_qkv_chain(b, m, nb, ps_slice):
                lhs = xt_sb[:, :, b * 256 + 128 * m : b * 256 + 128 * (m + 1)]
                for k in range(8):
                    nc.tensor.matmul(
                        ps_slice,
                        lhsT=lhs[:, k, :],
                        rhs=wq_sb[:, k, 512 * nb : 512 * (nb + 1)],
                        start=(k == 0),
                        stop=(k == 7),
                    )
                if b < 2:
                    # wide staging: data cols 0:64 + zero pad 64:128 so the
                    # xbar transpose readback sees defined data. The pad is
                    # memset only on the first rotation of each pool buffer.
                    ybuf = ybp.tile([P, 8, P], BF16, tag="ybw")
                    nc.vector.memset(ybuf[:, :, 64:128], 0.0)
                    nc.vector.tensor_tensor(
                        ybuf[:, :, 0:64],
                        ps_slice.rearrange("p (j d) -> p j d", d=64),
                        bq_sb[:, 512 * nb : 512 * (nb + 1)].rearrange(
                            "p (j d) -> p j d", d=64
                        ),
                        ALU.add,
                    )
                    dst = (yq_v if b == 0 else yk_v)[
                        128 * m : 128 * (m + 1), 8 * nb : 8 * (nb + 1), :
                    ]
                    nc.sync.dma_start(dst, ybuf[:])
                else:
                    ybuf = ybp.tile([P, 512], BF16, tag="yb")
                    nc.vector.tensor_tensor(
                        ybuf[:], ps_slice, bq_sb[:, 512 * nb : 512 * (nb + 1)], ALU.add
                    )
                    nc.sync.dma_start(
                        yv_v[128 * m : 128 * (m + 1), 8 * nb : 8 * (nb + 1), :],
                        ybuf[:].rearrange("p (j d) -> p j d", d=64),
                    )

            # m0 lead-in: round-robin chains over the 4 score slots + yps bank
            # (attention hasn't started; those banks are free).
            lead_chains = [(b, 0, nb) for nb in range(6) for b in (0, 1)]
            for i, (b, m, nb) in enumerate(lead_chains):
                slot = i % 5
                if slot < 4:
                    emit_qkv_chain(b, m, nb, sc[:, slot, :])
                else:
                    yps = yps_p.tile([P, 512], F32, tag="yps")
                    emit_qkv_chain(b, m, nb, yps[:])

            # remaining chains ride the shared yps bank, interleaved into the
            # attention windows per this static schedule (constraints: b0m1
            # complete before fe[3] (end of h1 works since fe[3] also needs
            # b1m1, done h1), b2m1 complete before vx(3) at h2-u24):
            pieces = {
                0: [(0, 1, nb) for nb in range(6)],
                1: [(1, 1, nb) for nb in range(6)],
                2: [(2, 1, nb) for nb in range(6)],
                3: [],
                4: [],
                5: [],
            }

            def emit_piece(pc):
                b, m, nb = pc
                yps = yps_p.tile([P, 512], F32, tag="yps")
                emit_qkv_chain(b, m, nb, yps[:])

            # ---------------- attention per head ----------------
            def emit_frontend(l):
                qT = qkp.tile([P, SEQ], BF16, tag="qk", name=f"qT{l}")
                nc.sync.dma_start(qT[:], yq[SEQ * l : SEQ * (l + 1), :], transpose=True)
                kT = qkp.tile([P, SEQ], BF16, tag="qk", name=f"kT{l}")
                nc.sync.dma_start(kT[:], yk[SEQ * l : SEQ * (l + 1), :], transpose=True)
                q8 = q8p.tile([64, SEQ], FP8, tag="q8", name=f"q8_{l}")
                nc.vector.tensor_copy(out=q8[:], in_=qT[0:64, :])
                k8 = q8p.tile([64, SEQ], FP8, tag="q8", name=f"k8_{l}")
                nc.vector.tensor_copy(out=k8[:], in_=kT[0:64, :])
                return q8, k8

            def emit_vx(l):
                vx = vxp.tile([P, 16, 65], BF16, name=f"vx{l}", tag="vx")
                nc.vector.memset(vx[:, :, 64:65], 1.0)
                nc.sync.dma_start(
                    vx[:, :, 0:64],
                    yv[SEQ * l : SEQ * (l + 1), :].rearrange(
                        "(so p) d -> p so d", p=P
                    ),
                )
                return vx

            def dup2(ap, n):
                return ap.rearrange("p (o t) -> p o t", o=1).to_broadcast((64, 2, n))

            def emit_scores_unit(l, q8, k8, tt, h, expT):
                # unit = (tt, h): scores [128 t, 1024 s], s0 = 1024h
                pair = 2 * ((2 * tt + h) % 2)
                lhsT = dup2(k8[:, 128 * tt : 128 * (tt + 1)], 128)
                for half in range(2):
                    s0 = 1024 * h + 512 * half
                    nc.tensor.matmul(
                        sc[:, pair + half, :],
                        lhsT=lhsT,
                        rhs=dup2(q8[:, s0 : s0 + 512], 512),
                        start=True,
                        stop=True,
                        perf_mode=PM.DoubleRow,
                    )
                nc.scalar.activation(
                    expT[:, 1024 * h : 1024 * (h + 1)],
                    sc[:, pair : pair + 2, :],
                    AF.Exp,
                    scale=ACT_SCALE,
                )

            def emit_ctx(l, vx, tt, expT):
                # PSUM start=True zeroes a whole 2KiB bank, so emit exactly one
                # start (first mm into the bank this head) and one stop (last
                # mm) per bank; intermediate mms rely on has_written bits.
                for j in range(16):
                    lhsT = expT.rearrange("p (i j) -> p j i", j=16)[:, j, :]
                    nc.tensor.matmul(
                        ctxps[:, j, :],
                        lhsT=lhsT,
                        rhs=vx[:, tt, 0:64],
                        start=(tt == 0 and j % 8 == 0),
                        stop=(tt == 15 and j % 8 == 7),
                    )
                    nc.tensor.matmul(
                        denps[:, j : j + 1],
                        lhsT=lhsT,
                        rhs=vx[:, tt, 64:65],
                        start=(tt == 0 and j == 0),
                        stop=(tt == 15 and j == 15),
                    )

            def emit_normalize(l):
                rec = rsp.tile([P, 16], F32, tag="rs")
                nc.vector.reciprocal(rec[:], denps[:])
                x_sb = xsp.tile([P, 16, 64], BF16, tag="xs", name=f"x{l}")
                # read each bank's j%8==0 slice last: the next head's bank-
                # clearing start=True matmul only carries a WAR against that
                # slice, so it must be the final read of the bank.
                for j in [1, 2, 3, 4, 5, 6, 7, 0, 9, 10, 11, 12, 13, 14, 15, 8]:
                    nc.vector.tensor_scalar(
                        x_sb[:, j, :],
                        ctxps[:, j, :],
                        rec[:, j : j + 1],
                        None,
                        ALU.mult,
                    )
                nc.sync.dma_start(xd[l][:], x_sb.rearrange("p j d -> p (j d)"))

            def emit_outproj(l):
                xt2 = xtp.tile([P, 8, P], BF16, tag="xt2", name=f"xt2_{l}")
                for c in range(8):
                    nc.sync.dma_start(
                        xt2[:, c, :],
                        xd[l][:, 128 * c : 128 * (c + 1)],
                        transpose=True,
                    )
                stg = stgp.tile([P, 8, P], F32, tag="stg", name=f"stg{l}")
                for m in range(8):
                    ops = yps_p.tile([P, 512], F32, tag="yps", name=f"ops{l}_{m}")
                    for c in range(8):
                        nc.tensor.matmul(
                            ops[:, 0:128],
                            lhsT=wo_sb[:, c, 128 * m : 128 * (m + 1)],
                            rhs=xt2[:, c, :],
                            start=(c == 0),
                            stop=(c == 7),
                        )
                    nc.vector.tensor_scalar(
                        stg[:, m, :], ops[:, 0:128], bo_sb[:, m : m + 1], None, ALU.add
                    )
                nc.sync.dma_start(
                    out_e.rearrange("(m p) r -> p m r", p=P)[
                        :, :, 128 * l : 128 * (l + 1)
                    ],
                    stg[:],
                )

            # ---------------- emission schedule ----------------
            fe = {}
            fe[0] = emit_frontend(0)
            for nb in range(6):  # b2 m0 before vx(0)
                emit_piece((2, 0, nb))
            vx_t = {0: emit_vx(0)}
            fe[1] = emit_frontend(1)

            for l in range(6):
                q8, k8 = fe[l]
                vx = vx_t[l]
                expTs = {}
                rest = list(pieces[l])
                unit_jobs = {}
                for i, pc in enumerate(rest):
                    unit_jobs.setdefault(2 + 3 * i, []).append(pc)
                for u in range(32):
                    tt, h = divmod(u, 2)
                    if h == 0:
                        expTs[tt] = expp.tile(
                            [P, SEQ], BF16, tag="expT", name=f"expT{l}_{tt}"
                        )
                    emit_scores_unit(l, q8, k8, tt, h, expTs[tt])
                    for pc in unit_jobs.get(u, []):
                        emit_piece(pc)
                    if h == 1 and tt >= 1:
                        emit_ctx(l, vx, tt - 1, expTs.pop(tt - 1))
                    # prefetch next head's frontend mid-loop; fe[3] needs
                    # b1m1 (emitted during h1) so it is emitted at h1 end.
                    if u == 16 and l + 2 <= 5 and l != 1:
                        fe[l + 2] = emit_frontend(l + 2)
                    if u == 24 and l + 1 <= 5:
                        vx_t[l + 1] = emit_vx(l + 1)
                emit_ctx(l, vx, 15, expTs.pop(15))
                if l == 1:
                    fe[3] = emit_frontend(3)
                emit_normalize(l)
                if l >= 1:
                    emit_outproj(l - 1)
            emit_outproj(5)

    nc.finalize()
    return nc


def _get_nc():
    if "nc" not in _NC_CACHE:
        _NC_CACHE["nc"] = _build()
    return _NC_CACHE["nc"]


def kernel(inputs, W_qkv, b_qkv, W_out, b_out, _trace=False, _trace_kwargs=None):
    bf = ml_dtypes.bfloat16
    x = np.asarray(inputs, dtype=np.float32)
    Wq = np.asarray(W_qkv, dtype=np.float32)
    bq = np.asarray(b_qkv, dtype=np.float32)
    Wo = np.asarray(W_out, dtype=np.float32)
    bo = np.asarray(b_out, dtype=np.float32)

    wq_s = np.ascontiguousarray(Wq.reshape(8, P, 3072).transpose(1, 0, 2)).astype(bf)
    wo_s = np.ascontiguousarray(Wo.reshape(8, P, 1024).transpose(1, 0, 2)).astype(bf)
    bq_s = np.ascontiguousarray(np.broadcast_to(bq[None, :], (P, 3072))).astype(
        np.float32
    )
    bo_s = np.ascontiguousarray(bo.reshape(8, P).T).astype(np.float32)

    in_maps = []
    for c in range(N_CORES):
        xc = x[:, ROWS * c : ROWS * (c + 1), :]  # [3, 256, 1024]
        xt = (
            xc.transpose(2, 0, 1)
            .reshape(1024, 768)
            .reshape(8, P, 768)
            .transpose(1, 0, 2)
        )
        in_maps.append(
            {
                "xt": np.ascontiguousarray(xt).astype(bf),
                "wq": wq_s,
                "bq": bq_s,
                "wo": wo_s,
                "bo": bo_s,
            }
        )

    nc = _get_nc()
    kw = {}
    if _trace:
        kw["trace"] = True
        if _trace_kwargs:
            kw.update(_trace_kwargs)
    res = run_bass_kernel_spmd(nc, in_maps, core_ids=list(range(N_CORES)), **kw)
    outs = res.results

    out = np.empty((6144, 1024), dtype=np.float32)
    for c in range(N_CORES):
        out[768 * c : 768 * (c + 1), :] = np.asarray(
            outs[c]["outt"], dtype=np.float32
        ).T
    if _trace:
        kernel.last_result = res
    return out.reshape(3, SEQ, H)
